# revision 3
# baseline (speedup 1.0000x reference)
"""Multi-head attention (QKV proj + RoPE + softmax attention) on 8 Trainium2
NeuronCores, tensor-parallel over heads (2 heads per core).

Contract: kernel(**inputs) takes the FULL unsharded inputs (numpy/jax arrays,
shapes hardcoded below) and returns the FULL [B, S, H] output.
"""

from contextlib import ExitStack

import numpy as np

B, S, H = 2, 2048, 2048
NH, D = 16, 128
ROPE_BASE = 10000.0
NCORES = 8
HPC = NH // NCORES          # heads per core
CH = HPC * D                # output channels per core
BS = B * S                  # flattened tokens
KT = H // 128               # contraction k-tiles
NCH = BS // 512             # 512-wide token chunks
SKT = S // 128              # score k-tiles per sequence
SQC = S // 512              # query chunks per sequence

LAST_RESULT = None          # BassKernelResults of the most recent run (for test.py)


def _build_nc(with_bias):
    import concourse.mybir as mybir
    import concourse.tile as tile
    from concourse import bacc
    from concourse.masks import make_identity

    F32 = mybir.dt.float32
    F32R = mybir.dt.float32r
    AF = mybir.ActivationFunctionType
    ALU = mybir.AluOpType
    ISCALE = float(1.0 / np.sqrt(D))

    nc = bacc.Bacc("TRN2", debug=False, enable_partition_id=False)

    hsT_d = nc.dram_tensor("hsT", [H, BS], F32R, kind="ExternalInput").ap()
    wT_d = {
        p: nc.dram_tensor(f"w{p}T", [H, CH], F32R, kind="ExternalInput").ap()
        for p in "qkv"
    }
    b_d = {
        p: nc.dram_tensor(f"b{p}", [1, CH], F32R, kind="ExternalInput").ap()
        for p in "qkv"
    }
    cos_d = nc.dram_tensor("cosT", [D, S], F32, kind="ExternalInput").ap()
    sin_d = nc.dram_tensor("sinT", [D, S], F32, kind="ExternalInput").ap()
    out_d = nc.dram_tensor("out", [BS, CH], F32, kind="ExternalOutput").ap()

    with tile.TileContext(nc) as tc, ExitStack() as ctx:
        # ---- persistent state (lives across both phases) ----
        persist = ctx.enter_context(tc.tile_pool(name="persist", bufs=1))
        qT = [persist.tile([128, BS], F32R, tag=f"qT{m}", name=f"qT{m}") for m in range(HPC)]
        kTt = [persist.tile([128, BS], F32R, tag=f"kT{m}", name=f"kT{m}") for m in range(HPC)]
        vN = [persist.tile([128, BS // 128, D], F32R, tag=f"v{m}", name=f"vn{m}") for m in range(HPC)]

        consts = ctx.enter_context(tc.tile_pool(name="consts", bufs=1))
        ident = consts.tile([128, 128], F32, tag="ident")
        make_identity(nc, ident)
        ones_mat = consts.tile([128, 128], F32, tag="ones_mat")
        nc.vector.memset(ones_mat, 1.0)
        if with_bias:
            ones_row = consts.tile([1, 512], F32, tag="ones_row")
            nc.vector.memset(ones_row, 1.0)
            ones_col = consts.tile([1, 128], F32, tag="ones_col")
            nc.vector.memset(ones_col, 1.0)
            b_sb = {}
            for p in "qkv":
                b_sb[p] = consts.tile([1, CH], F32R, tag=f"b{p}", name=f"b{p}sb")
                nc.sync.dma_start(b_sb[p], b_d[p])

        # ================= Phase 1: QKV projections + RoPE =================
        with (
            tc.tile_pool(name="wpool", bufs=1) as wpool,
            tc.tile_pool(name="tabs", bufs=1) as tabs,
            tc.tile_pool(name="hstp", bufs=8) as hstp,
            tc.tile_pool(name="p1ps", bufs=1, space="PSUM") as p1ps,
            tc.tile_pool(name="ropet", bufs=3) as ropet,
        ):
            w_sb = {}
            for p in "qkv":
                w_sb[p] = wpool.tile([128, KT, CH], F32R, tag=f"w{p}", name=f"w{p}sb")
                nc.sync.dma_start(w_sb[p], wT_d[p].rearrange("(k p) c -> p k c", p=128))
            cos_sb = tabs.tile([D, S], F32, tag="cos")
            sin_sb = tabs.tile([D, S], F32, tag="sin")
            nc.sync.dma_start(cos_sb, cos_d)
            nc.sync.dma_start(sin_sb, sin_d)

            hsT_r = hsT_d.rearrange("(k p) t -> p k t", p=128)

            for n in range(NCH):
                tok = slice(n * 512, (n + 1) * 512)
                pos = slice((n % SQC) * 512, (n % SQC + 1) * 512)
                hs_t = [hstp.tile([128, 512], F32R, tag="hs", name=f"hs{k}") for k in range(KT)]
                for k in range(KT):
                    nc.sync.dma_start(hs_t[k], hsT_r[:, k, tok])

                qk_ps = {
                    (p, m): p1ps.tile([128, 512], F32, tag=f"{p}{m}", name=f"ps{p}{m}")
                    for p in "qk"
                    for m in range(HPC)
                }
                v_ps = [p1ps.tile([128, CH], F32, tag=f"v{u}", name=f"psv{u}") for u in range(4)]
                for k in range(KT):
                    for p in "qk":
                        for m in range(HPC):
                            nc.tensor.matmul(
                                qk_ps[(p, m)],
                                w_sb[p][:, k, m * 128:(m + 1) * 128],
                                hs_t[k],
                                start=(k == 0),
                                stop=(k == KT - 1) and not with_bias,
                            )
                    for u in range(4):
                        nc.tensor.matmul(
                            v_ps[u],
                            hs_t[k][:, u * 128:(u + 1) * 128],
                            w_sb["v"][:, k, :],
                            start=(k == 0),
                            stop=(k == KT - 1) and not with_bias,
                        )
                if with_bias:
                    for p in "qk":
                        for m in range(HPC):
                            nc.tensor.matmul(
                                qk_ps[(p, m)],
                                b_sb[p][:, m * 128:(m + 1) * 128],
                                ones_row.bitcast(F32R),
                                start=False,
                                stop=True,
                            )
                    for u in range(4):
                        nc.tensor.matmul(
                            v_ps[u], ones_col.bitcast(F32R), b_sb["v"], start=False, stop=True
                        )

                # RoPE on q/k (drains qk psum), plain copy for v
                for p, dst in (("q", qT), ("k", kTt)):
                    for m in range(HPC):
                        ps = qk_ps[(p, m)]
                        t1 = ropet.tile([128, 512], F32, tag="t1")
                        nc.vector.tensor_tensor(t1, ps, cos_sb[:, pos], op=ALU.mult)
                        t2 = ropet.tile([128, 512], F32, tag="t2")
                        nc.vector.scalar_tensor_tensor(
                            t2[0:64], ps[64:128], -1.0, sin_sb[0:64, pos],
                            op0=ALU.mult, op1=ALU.mult,
                        )
                        nc.vector.scalar_tensor_tensor(
                            t2[64:128], ps[0:64], 1.0, sin_sb[64:128, pos],
                            op0=ALU.mult, op1=ALU.mult,
                        )
                        nc.vector.tensor_tensor(dst[m][:, tok], t1, t2, op=ALU.add)
                for u in range(4):
                    st = n * 4 + u
                    for m in range(HPC):
                        nc.vector.tensor_copy(
                            vN[m][:, st, :], v_ps[u][:, m * 128:(m + 1) * 128]
                        )

        # ================= Phase 2: attention =================
        with (
            tc.tile_pool(name="epool", bufs=20) as epool,
            tc.tile_pool(name="opool", bufs=4) as opool,
            tc.tile_pool(name="stps", bufs=3, space="PSUM") as stps,
            tc.tile_pool(name="otps", bufs=2, space="PSUM") as otps,
            tc.tile_pool(name="dnps", bufs=2, space="PSUM") as dnps,
            tc.tile_pool(name="trps", bufs=1, space="PSUM") as trps,
        ):
            for m in range(HPC):
                for b in range(B):
                    for c in range(SQC):
                        sq = slice(b * S + c * 512, b * S + (c + 1) * 512)
                        ot_ps = otps.tile([128, 512], F32, tag="ot")
                        dn_ps = dnps.tile([128, 512], F32, tag="dn")
                        e_t = []
                        for sk in range(SKT):
                            st_ps = stps.tile([128, 512], F32, tag="st")
                            nc.tensor.matmul(
                                st_ps,
                                kTt[m][:, b * S + sk * 128: b * S + (sk + 1) * 128],
                                qT[m][:, sq],
                                start=True,
                                stop=True,
                            )
                            e_sb = epool.tile([128, 512], F32R, tag="e")
                            nc.scalar.activation(e_sb, st_ps, AF.Exp, scale=ISCALE)
                            e_t.append(e_sb)
                        for sk in range(SKT):
                            nc.tensor.matmul(
                                ot_ps,
                                vN[m][:, b * SKT + sk, :],
                                e_t[sk],
                                start=(sk == 0),
                                stop=(sk == SKT - 1),
                            )
                            nc.tensor.matmul(
                                dn_ps,
                                ones_mat.bitcast(F32R),
                                e_t[sk],
                                start=(sk == 0),
                                stop=(sk == SKT - 1),
                            )
                        rd = opool.tile([128, 512], F32, tag="rd")
                        nc.vector.reciprocal(rd, dn_ps)
                        otn = opool.tile([128, 512], F32, tag="otn")
                        nc.vector.tensor_tensor(otn, ot_ps, rd, op=ALU.mult)
                        for blk in range(4):
                            tr_ps = trps.tile([128, 128], F32, tag="tr")
                            nc.tensor.transpose(
                                tr_ps, otn[:, blk * 128:(blk + 1) * 128], ident
                            )
                            o_sb = opool.tile([128, 128], F32, tag="o")
                            nc.vector.tensor_copy(o_sb, tr_ps)
                            r0 = b * S + c * 512 + blk * 128
                            nc.sync.dma_start(
                                out_d[r0:r0 + 128, m * 128:(m + 1) * 128], o_sb
                            )

    nc.compile()
    return nc


def _rope_tables():
    inv_freq = 1.0 / (ROPE_BASE ** (np.arange(0, D, 2, dtype=np.float64) / D))
    pos = np.arange(S, dtype=np.float64)
    ang = pos[:, None] * inv_freq[None, :]          # [S, D/2]
    emb = np.concatenate([ang, ang], axis=-1)       # [S, D]
    cosT = np.ascontiguousarray(np.cos(emb).T.astype(np.float32))  # [D, S]
    sinT = np.ascontiguousarray(np.sin(emb).T.astype(np.float32))
    return cosT, sinT


def kernel(hidden_states, Wq, bq, Wk, bk, Wv, bv):
    global LAST_RESULT
    from concourse.bass_utils import run_bass_kernel_spmd

    hs = np.asarray(hidden_states, dtype=np.float32).reshape(BS, H)
    Wq = np.asarray(Wq, dtype=np.float32)
    Wk = np.asarray(Wk, dtype=np.float32)
    Wv = np.asarray(Wv, dtype=np.float32)
    bq = np.asarray(bq, dtype=np.float32)
    bk = np.asarray(bk, dtype=np.float32)
    bv = np.asarray(bv, dtype=np.float32)

    with_bias = bool(np.any(bq) or np.any(bk) or np.any(bv))
    nc = _build_nc(with_bias)

    hsT = np.ascontiguousarray(hs.T)                # [H, BS]
    cosT, sinT = _rope_tables()

    in_maps = []
    for c in range(NCORES):
        ch = slice(c * CH, (c + 1) * CH)
        m = {
            "hsT": hsT,
            "wqT": np.ascontiguousarray(Wq[ch, :].T),
            "wkT": np.ascontiguousarray(Wk[ch, :].T),
            "wvT": np.ascontiguousarray(Wv[ch, :].T),
            "cosT": cosT,
            "sinT": sinT,
        }
        if with_bias:
            m["bq"] = np.ascontiguousarray(bq[None, ch])
            m["bk"] = np.ascontiguousarray(bk[None, ch])
            m["bv"] = np.ascontiguousarray(bv[None, ch])
        else:
            z = np.zeros((1, CH), dtype=np.float32)
            m["bq"] = m["bk"] = m["bv"] = z
        in_maps.append(m)

    res = run_bass_kernel_spmd(nc, in_maps, core_ids=list(range(NCORES)))
    LAST_RESULT = res

    full = np.concatenate([r["out"] for r in res.results], axis=1)  # [BS, H]
    return full.reshape(B, S, H)
